# revision 19
# baseline (speedup 1.0000x reference)
"""Trainium2 Bass kernel for nn_CSG2A_net (gnn_message_passing).

Math (algebraically identical to the reference, never materializes the
[B,G,G] score tensor):
  CCE:  h = relu(node_feat @ W1); w = adj*exp(-dist)
        g[b,m] = sum_n mask[b,n] * w[b,n,m]
        pooled[b,d] = (sum_m g[b,m] h[b,m,d]) / clip(sum_n mask[b,n], 1)
        comp = pooled @ W2 + dose @ w_dose + time @ w_time
  score.sum(-1)[b,g] = q[b,g,:] . (sum_k q[b,k,:]) / sqrt(H)
    with q[b,g,:] = b_gex[b,g] w_gex[g,:] + comp[b,g] w_comp[g,:]
    so  u = b_gex @ w_gex + comp @ w_comp          [B,H]
        A = u @ w_gex.T ; C = u @ w_comp.T         [B,G]
        ssum = (b_gex*A + comp*C) / sqrt(H)
  pred = b_gex * (ssum + ppi_adj.sum(-1))
  out  = relu(LN(pred)) @ W_ff

Sharding: data-parallel over batch across 8 cores (8 samples each).
ppi_adj is additionally ROW-SHARDED: each core loads a distinct 128-row
slice, reduces it to per-row sums on-chip, and a tiny (512B-per-rank)
AllGather distributes the full row-sum vector — this removes 7/8 of the
largest replicated load (3.8MB -> 0.5MB per core).  Other weights are
replicated; host-side prep only slices / transposes / pads / concats
(pure data movement) so small params arrive in a handful of DMAs.

On-chip layout is gene-major ([G-tile partitions x batch free]) so every
matmul contracts on the partition dim.  Big loads are spread across the
five engine DMA queues; FFN matmuls run as float32r for 4x PE rate.
"""

import numpy as np

import concourse.bass as bass
import concourse.mybir as mybir
import concourse.tile as tile
from concourse.bass_utils import run_bass_kernel_spmd
from concourse.masks import make_identity

F32 = mybir.dt.float32
F32R = mybir.dt.float32r
AF = mybir.ActivationFunctionType

G, H, NA, FEAT, CH = 978, 128, 50, 34, 64
B, NCORES = 64, 8
BL = B // NCORES  # per-core batch
LN_EPS = 1e-5
# gene-dim tiles: 7 x 128 + 82
GTS = [(i * 128, 128) for i in range(7)] + [(896, 82)]
NGT = len(GTS)
GP = 128 * NGT  # gene dim padded to 1024

_DMA_ZERO_WAIT = ("InstDMACopy", "InstDMATransposeAnt", "InstTriggeredCopy",
                  "InstCollectiveCompute")


def _split_excess_waits(nc):
    """walrus in this container accepts at most 1 inline sync-wait per
    instruction (0 for DMA/collective).  Move excess waits onto same-engine
    nops inserted immediately before the overloaded instruction."""

    def make_nop(engine):
        bi = nc.engines[engine].nop(nofuse=True)
        ins = bi.ins
        lst = nc.cur_bb.bb.instructions
        assert lst[-1] is ins
        lst.pop()
        return ins

    for bb in nc.main_func.blocks:
        lst = bb.instructions
        i = 0
        while i < len(lst):
            ins = lst[i]
            si = getattr(ins, "sync_info", None)
            waits = list(si.on_wait) if (si and si.on_wait) else []
            limit = 0 if type(ins).__name__ in _DMA_ZERO_WAIT else 1
            if len(waits) > limit:
                keep = waits[len(waits) - limit:] if limit else []
                excess = waits[: len(waits) - limit]
                si.on_wait = keep
                pos = i
                for w in excess:
                    nop = make_nop(ins.engine)
                    nop.sync_info = mybir.SyncInfo(on_wait=[w], on_update=[])
                    lst.insert(pos, nop)
                    pos += 1
                    i += 1
            i += 1


def build_nc():
    nc = bass.Bass(num_devices=NCORES)

    # ---- kernel I/O (per-core shapes; layouts prepped on host) ----
    b_gex = nc.dram_tensor("b_gex", [BL, G], F32, kind="ExternalInput")
    # mix: [0:34, 0:400] node_feat^T (f, b*n); [0:34, 400:464] W1;
    #      [0:50, 464:472] mask^T; [0, 472:480] dose^T; [1, 472:480] time^T
    mix = nc.dram_tensor("mix", [52, 480], F32, kind="ExternalInput")
    adjT = nc.dram_tensor("adjT", [NA, BL * NA], F32, kind="ExternalInput")
    distT = nc.dram_tensor("distT", [NA, BL * NA], F32, kind="ExternalInput")
    # aux_row rows: 0 = w_dose, 1 = w_time, 2 = ln_gamma, 3 = ln_beta
    aux_row = nc.dram_tensor("aux_row", [4, G], F32, kind="ExternalInput")
    # this core's 128-row slice of ppi_adj (zero-padded past row 978)
    ppi_slice = nc.dram_tensor("ppi_slice", [128, G], F32, kind="ExternalInput")
    W2 = nc.dram_tensor("W2", [CH, G], F32, kind="ExternalInput")
    w_gex_p = nc.dram_tensor("w_gex_p", [GP, H], F32, kind="ExternalInput")
    w_comp_p = nc.dram_tensor("w_comp_p", [GP, H], F32, kind="ExternalInput")
    W_ff = nc.dram_tensor("W_ff", [G, G], F32, kind="ExternalInput")

    out_pred = nc.dram_tensor("out_pred", [BL, G], F32, kind="ExternalOutput")
    out_comp = nc.dram_tensor("out_comp", [BL, G], F32, kind="ExternalOutput")

    inv_sqrt_h = 1.0 / float(np.sqrt(H))

    with tile.TileContext(nc) as tc:
        with (
            tc.tile_pool(name="const", bufs=1) as const,
            tc.tile_pool(name="sb", bufs=1) as sb,
            tc.tile_pool(name="work", bufs=6) as work,
            tc.tile_pool(name="pacc", bufs=1, space="PSUM") as pacc,
            tc.tile_pool(name="pcyc", bufs=5, space="PSUM") as pcyc,
            tc.tile_pool(name="dram", bufs=1, space="DRAM") as dram,
        ):
            ident = const.tile([128, 128], F32)
            make_identity(nc, ident[:])
            ones_col = const.tile([128, 1], F32)   # lhsT for col-sums
            nc.vector.memset(ones_col[:], 1.0)
            ones_row = const.tile([1, 128], F32)   # lhsT for partition-bcast
            nc.vector.memset(ones_row[:], 1.0)
            eps_t = const.tile([1, 1], F32)
            nc.vector.memset(eps_t[:], LN_EPS)
            junk_row = const.tile([1, 120], F32R)
            nc.vector.memset(junk_row[:], 1.0)

            _cyc_n = [0]

            def cyc(shape):
                _cyc_n[0] += 1
                return pcyc.tile(shape, F32, tag="cyc", name=f"cyc{_cyc_n[0]}")

            # ============ DMA queues ============
            # SP: ppi slice first (feeds the collective), then the big
            # gene-major weights; Pool/ACT/DVE take the small/medium loads;
            # PE takes one W_ff chunk before its compute stream starts.
            GH2 = 512  # ppi column split point
            ppi_sb = sb.tile([128, G], F32)
            nc.sync.dma_start(out=ppi_sb[:, 0:GH2], in_=ppi_slice[:, 0:GH2])
            auxr_sb = sb.tile([2, G], F32)
            nc.sync.dma_start(out=auxr_sb[:], in_=aux_row[0:2, :])
            gam_sb = sb.tile([1, G], F32)
            nc.sync.dma_start(out=gam_sb[:], in_=aux_row[2:3, :])
            bet_sb = sb.tile([1, G], F32)
            nc.sync.dma_start(out=bet_sb[:], in_=aux_row[3:4, :])

            wg_sb = sb.tile([128, NGT, H], F32)
            nc.sync.dma_start(out=wg_sb[:],
                              in_=w_gex_p[:, :].rearrange("(t p) h -> p t h", p=128))
            wc_sb = sb.tile([128, NGT, H], F32)
            nc.sync.dma_start(out=wc_sb[:],
                              in_=w_comp_p[:, :].rearrange("(t p) h -> p t h", p=128))

            W2_sb = sb.tile([CH, G], F32)
            nc.sync.dma_start(out=W2_sb[:], in_=W2[:, :])

            wff_sb = sb.tile([128, NGT, G], F32R)

            def wff_chunk(eng, c):
                eng.dma_start(
                    out=wff_sb[:, 2 * c:2 * c + 2, :],
                    in_=W_ff[256 * c:256 * (c + 1), :].bitcast(F32R).rearrange(
                        "(t p) k -> p t k", p=128))

            wff_chunk(nc.sync, 0)                   # rows   0:256
            wff_chunk(nc.sync, 1)                   # rows 256:512
            wff_chunk(nc.sync, 2)                   # rows 512:768

            # Pool: mix, then the prs collective chain
            mix_sb = sb.tile([52, 480], F32)
            nc.gpsimd.dma_start(out=mix_sb[:], in_=mix[:, :])

            # ACT: second ppi half, CCE inputs, activation-table warm, b_gex
            nc.scalar.dma_start(out=ppi_sb[:, GH2:G], in_=ppi_slice[:, GH2:G])
            adjT_sb = sb.tile([NA, BL * NA], F32)
            nc.scalar.dma_start(out=adjT_sb[:], in_=adjT[:, :])
            distT_sb = sb.tile([NA, BL * NA], F32)
            nc.scalar.dma_start(out=distT_sb[:], in_=distT[:, :])
            warm = sb.tile([1, 1], F32)
            nc.scalar.activation(warm[:], eps_t[:], AF.Ln)  # act-table preload
            b_nat = sb.tile([BL, G], F32)
            nc.scalar.dma_start(out=b_nat[:], in_=b_gex[:, :])

            # ppi row-sum (two halves on DVE/Pool) -> DRAM -> AllGather
            with tc.high_priority():
                rowsum = sb.tile([128, 1], F32)
                nc.vector.tensor_reduce(rowsum[:], ppi_sb[:],
                                        mybir.AxisListType.X, mybir.AluOpType.add)
                prs_in_d = dram.tile([128, 1], F32)
                nc.gpsimd.dma_start(out=prs_in_d[:], in_=rowsum[:])
                prs_out_d = dram.tile([NCORES, 128], F32)
                nc.gpsimd.collective_compute(
                    "AllGather",
                    mybir.AluOpType.bypass,
                    replica_groups=[list(range(NCORES))],
                    ins=[prs_in_d.opt()],
                    outs=[prs_out_d.opt()],
                )
                prs_pt = sb.tile([128, NGT], F32)
                nc.sync.dma_start(out=prs_pt[:],
                                  in_=prs_out_d[:].rearrange("t p -> p t"))
            # W_ff tail rows 768:896 + 896:978 ride Pool after the AG trigger
            nc.gpsimd.dma_start(out=wff_sb[:, 6, :],
                                in_=W_ff[768:896, :].bitcast(F32R))
            nc.gpsimd.dma_start(out=wff_sb[:82, 7, :],
                                in_=W_ff[896:G, :].bitcast(F32R))

            # ============ CCE ============
            maskT = mix_sb[0:NA, 464:472]
            dtT = mix_sb[0:2, 472:480]  # row 0 dose^T, row 1 time^T

            hT_ps = cyc([CH, BL * NA])
            nc.tensor.matmul(hT_ps[:], mix_sb[0:FEAT, 400:464].bitcast(F32R),
                             mix_sb[0:FEAT, 0:400].bitcast(F32R),
                             start=True, stop=True)
            hT = sb.tile([CH, BL * NA], F32)
            nc.scalar.activation(hT[:], hT_ps[:], AF.Relu)

            wmsg = sb.tile([NA, BL * NA], F32)
            nc.scalar.activation(wmsg[:], distT_sb[:], AF.Exp, scale=-1.0)
            nc.vector.tensor_mul(wmsg[:], wmsg[:], adjT_sb[:])

            g_ps = cyc([1, BL * NA])
            for b in range(BL):
                nc.tensor.matmul(g_ps[:, b * NA:(b + 1) * NA],
                                 maskT[:, b:b + 1], wmsg[:, b * NA:(b + 1) * NA],
                                 start=True, stop=True)
            gb_ps = cyc([CH, BL * NA])
            g_sb = sb.tile([1, BL * NA], F32)
            nc.vector.tensor_copy(g_sb[:], g_ps[:])
            nc.tensor.matmul(gb_ps[:], ones_row[:1, :CH].bitcast(F32R),
                             g_sb[:].bitcast(F32R), start=True, stop=True)

            prod = sb.tile([CH, BL * NA], F32)
            nc.vector.tensor_mul(prod[:], hT[:], gb_ps[:])
            pooled_raw = sb.tile([CH, BL], F32)
            nc.vector.tensor_reduce(pooled_raw[:], prod[:].rearrange("d (b n) -> d b n", b=BL),
                                    mybir.AxisListType.X, mybir.AluOpType.add)

            ms_ps = cyc([1, BL])
            nc.tensor.matmul(ms_ps[:], ones_col[:NA, :], maskT[:], start=True, stop=True)
            ms_sb = sb.tile([1, BL], F32)
            nc.vector.tensor_scalar_max(ms_sb[:], ms_ps[:], 1.0)
            rms = sb.tile([1, BL], F32)
            nc.vector.reciprocal(rms[:], ms_sb[:])
            rb_ps = cyc([CH, BL])
            nc.tensor.matmul(rb_ps[:], ones_row[:1, :CH], rms[:], start=True, stop=True)
            pooledT = sb.tile([CH, BL], F32)
            nc.vector.tensor_mul(pooledT[:], pooled_raw[:], rb_ps[:])

            # b_gex transposed to gene-major via PE; 4 transposes share one
            # PSUM bank -> one batched copy out
            bgT = sb.tile([128, NGT, BL], F32)
            for half in range(2):
                bg_ps = cyc([128, 4, BL])
                for j in range(4):
                    gt = half * 4 + j
                    gs, gn = GTS[gt]
                    nc.tensor.transpose(bg_ps[:gn, j, :], b_nat[:, gs:gs + gn],
                                        ident[:BL, :BL])
                if half == 0:
                    nc.scalar.copy(bgT[:, 0:4, :], bg_ps[:])
                else:
                    nc.scalar.copy(bgT[:, 4:7, :], bg_ps[:, 0:3, :])
                    nc.scalar.copy(bgT[:82, 7, :], bg_ps[:82, 3, :])

            # comp.T per gene tile (+ comp output)
            compT = sb.tile([128, NGT, BL], F32)  # [p, gt, b]
            comp_out = sb.tile([BL, G], F32)
            for half in range(2):
                cT_ps = cyc([128, 4, BL])
                for j in range(4):
                    gt = half * 4 + j
                    gs, gn = GTS[gt]
                    nc.tensor.matmul(cT_ps[:gn, j, :], W2_sb[:, gs:gs + gn], pooledT[:],
                                     start=True, stop=False)
                    nc.tensor.matmul(cT_ps[:gn, j, :], auxr_sb[0:2, gs:gs + gn], dtT[:],
                                     start=False, stop=True)
                if half == 0:
                    nc.scalar.copy(compT[:, 0:4, :], cT_ps[:])
                else:
                    nc.scalar.copy(compT[:, 4:7, :], cT_ps[:, 0:3, :])
                    nc.scalar.copy(compT[:82, 7, :], cT_ps[:82, 3, :])
            for half in range(2):
                c8_ps = cyc([BL, 512])
                w0 = half * 512
                for j in range(4):
                    gt = half * 4 + j
                    gs, gn = GTS[gt]
                    nc.tensor.transpose(c8_ps[:, gs - w0:gs - w0 + gn],
                                        compT[:gn, gt, :], ident[:gn, :gn])
                wid = 512 if half == 0 else G - 512
                nc.scalar.copy(comp_out[:, w0:w0 + wid], c8_ps[:, :wid])
            nc.sync.dma_start(out=out_comp[:, :], in_=comp_out[:])

            # ============ attention-sum ============
            u_ps = pacc.tile([H, BL], F32, tag="u")
            for gt, (gs, gn) in enumerate(GTS):
                nc.tensor.matmul(u_ps[:], wg_sb[:gn, gt, :], bgT[:gn, gt, :],
                                 start=(gt == 0), stop=False)
            for gt, (gs, gn) in enumerate(GTS):
                nc.tensor.matmul(u_ps[:], wc_sb[:gn, gt, :], compT[:gn, gt, :],
                                 start=False, stop=(gt == NGT - 1))
            u_sb = sb.tile([H, BL], F32)
            nc.scalar.copy(u_sb[:], u_ps[:])

            # A/C + pred (gene-major); stats matmuls deferred so the PE
            # queue never stalls on the collective-gated pred tiles
            predT = sb.tile([128, NGT, BL], F32)
            sqT = sb.tile([128, NGT, BL], F32)
            t1T = sb.tile([128, NGT, BL], F32)
            t2T = sb.tile([128, NGT, BL], F32)
            wgcT_pair = None
            for gt, (gs, gn) in enumerate(GTS):
                if gt % 2 == 0:
                    gn1 = GTS[gt + 1][1]
                    wgc_ps = cyc([128, 4, 128])
                    nc.tensor.transpose(wgc_ps[:, 0, :gn], wg_sb[:gn, gt, :],
                                        ident[:gn, :gn])
                    nc.tensor.transpose(wgc_ps[:, 1, :gn], wc_sb[:gn, gt, :],
                                        ident[:gn, :gn])
                    nc.tensor.transpose(wgc_ps[:, 2, :gn1], wg_sb[:gn1, gt + 1, :],
                                        ident[:gn1, :gn1])
                    nc.tensor.transpose(wgc_ps[:, 3, :gn1], wc_sb[:gn1, gt + 1, :],
                                        ident[:gn1, :gn1])
                    wgcT_pair = work.tile([H, 4, 128], F32, tag="wgcT")
                    if gn1 == 128:
                        if (gt // 2) % 2 == 0:
                            nc.scalar.activation(
                                wgcT_pair[:].rearrange("p s h -> p (s h)"),
                                wgc_ps[:].rearrange("p s h -> p (s h)"),
                                AF.Copy, scale=inv_sqrt_h)
                        else:
                            nc.vector.tensor_scalar_mul(
                                wgcT_pair[:].rearrange("p s h -> p (s h)"),
                                wgc_ps[:].rearrange("p s h -> p (s h)"),
                                inv_sqrt_h)
                    else:
                        nc.vector.tensor_scalar_mul(
                            wgcT_pair[:, 0:2, :].rearrange("p s h -> p (s h)"),
                            wgc_ps[:, 0:2, :].rearrange("p s h -> p (s h)"),
                            inv_sqrt_h)
                        nc.vector.tensor_scalar_mul(
                            wgcT_pair[:, 2:4, :gn1],
                            wgc_ps[:, 2:4, :gn1],
                            inv_sqrt_h)
                wgcT = wgcT_pair
                so = (gt % 2) * 2

                A_ps = cyc([128, BL])
                nc.tensor.matmul(A_ps[:gn, :], wgcT[:, so, :gn], u_sb[:],
                                 start=True, stop=True)
                C_ps = cyc([128, BL])
                nc.tensor.matmul(C_ps[:gn, :], wgcT[:, so + 1, :gn], u_sb[:],
                                 start=True, stop=True)

                nc.vector.tensor_mul(t1T[:gn, gt, :], bgT[:gn, gt, :], A_ps[:gn, :])
                nc.vector.tensor_mul(t2T[:gn, gt, :], compT[:gn, gt, :], C_ps[:gn, :])
                nc.vector.tensor_add(t1T[:gn, gt, :], t1T[:gn, gt, :],
                                     t2T[:gn, gt, :])
                # pred = b_gex * (ssum + prs)
                nc.vector.scalar_tensor_tensor(predT[:gn, gt, :], t1T[:gn, gt, :],
                                               prs_pt[:gn, gt:gt + 1], bgT[:gn, gt, :],
                                               op0=mybir.AluOpType.add,
                                               op1=mybir.AluOpType.mult)
                nc.gpsimd.tensor_mul(sqT[:gn, gt, :], predT[:gn, gt, :],
                                     predT[:gn, gt, :])

            # LN stats (all after pred so PE in-order queue can't stall early)
            stats_ps = pacc.tile([1, 2, BL], F32, tag="sxx")
            stats_x = stats_ps[:, 0, :]
            stats_x2 = stats_ps[:, 1, :]
            for gt, (gs, gn) in enumerate(GTS):
                nc.tensor.matmul(stats_x, ones_col[:gn, :], predT[:gn, gt, :],
                                 start=(gt == 0), stop=(gt == NGT - 1))
            for gt, (gs, gn) in enumerate(GTS):
                nc.tensor.matmul(stats_x2, ones_col[:gn, :], sqT[:gn, gt, :],
                                 start=(gt == 0), stop=(gt == NGT - 1))

            # ============ LayerNorm + ReLU ============
            mu = sb.tile([1, BL], F32)
            nc.vector.tensor_scalar_mul(mu[:], stats_x, 1.0 / G)
            mu2 = sb.tile([1, BL], F32)
            nc.vector.tensor_mul(mu2[:], mu[:], mu[:])
            var = sb.tile([1, BL], F32)
            nc.vector.scalar_tensor_tensor(var[:], stats_x2, 1.0 / G, mu2[:],
                                           op0=mybir.AluOpType.mult,
                                           op1=mybir.AluOpType.subtract)
            # rstd = exp(-0.5*ln(var+eps)): Ln/Exp live in the same ACT
            # table as Relu/Copy, so no mid-kernel table reload
            lnv = sb.tile([1, BL], F32)
            nc.scalar.activation(lnv[:], var[:], AF.Ln, bias=eps_t[:1, 0:1])
            rstd = sb.tile([1, BL], F32)
            nc.scalar.activation(rstd[:], lnv[:], AF.Exp, scale=-0.5)
            # xn = relu(pred*P + Q) with P = gamma (x) rstd and
            # Q = beta (x) 1 - gamma (x) (mu*rstd)  (rank-1 PE broadcasts)
            negmr = sb.tile([1, BL], F32)
            nc.vector.scalar_tensor_tensor(negmr[:], mu[:], -1.0, rstd[:],
                                           op0=mybir.AluOpType.mult,
                                           op1=mybir.AluOpType.mult)
            xn = sb.tile([128, NGT, BL], F32R)
            for gt, (gs, gn) in enumerate(GTS):
                PQ = cyc([128, 2, BL])
                nc.tensor.matmul(PQ[:gn, 0, :], gam_sb[0:1, gs:gs + gn], rstd[:],
                                 start=True, stop=True)
                nc.tensor.matmul(PQ[:gn, 1, :], bet_sb[0:1, gs:gs + gn],
                                 ones_row[:1, 0:BL], start=True, stop=False)
                nc.tensor.matmul(PQ[:gn, 1, :], gam_sb[0:1, gs:gs + gn], negmr[:],
                                 start=False, stop=True)
                eng = nc.vector if gt % 2 == 0 else nc.gpsimd
                xm = work.tile([128, BL], F32, tag="xm")
                eng.tensor_mul(xm[:gn, :], predT[:gn, gt, :], PQ[:gn, 0, :])
                eng.tensor_add(xm[:gn, :], xm[:gn, :], PQ[:gn, 1, :])
                eng.tensor_scalar_max(xn[:gn, gt, :], xm[:gn, :], 0.0)

            # ============ FFN (float32r for 4x PE rate) ============
            NSPLIT = [(0, 512), (512, 466)]
            o_ps = [pcyc.tile([BL, n], F32, tag="cyc", name=f"o_ps{i}")
                    for i, (s, n) in enumerate(NSPLIT)]
            for kt, (ks, kn) in enumerate(GTS):
                for i, (ns, nn) in enumerate(NSPLIT):
                    nc.tensor.matmul(o_ps[i][:],
                                     xn[:kn, kt, :],
                                     wff_sb[:kn, kt, ns:ns + nn],
                                     start=(kt == 0), stop=(kt == NGT - 1))
            pred_out = sb.tile([BL, G], F32)
            nc.scalar.copy(pred_out[:, 0:512], o_ps[0][:])
            nc.vector.tensor_copy(pred_out[:, 512:G], o_ps[1][:])
            nc.sync.dma_start(out=out_pred[:, 0:512], in_=pred_out[:, 0:512])
            nc.sync.dma_start(out=out_pred[:, 512:G], in_=pred_out[:, 512:G])

            # PE clock-warm fillers: gated on the AllGather completion, at
            # the lowest scheduler priority so they only occupy PE idle gaps
            # between the collective's return and the FFN, keeping the PE
            # p-state ramped for the FFN matmuls.
            prs_mini = sb.tile([1, 128], F32)
            nc.scalar.dma_start(out=prs_mini[:], in_=prs_out_d[0:1, :])
            fill_ps = pacc.tile([1, 120], F32, tag="fill")
            for _ in range(16):
                nc.tensor.matmul(fill_ps[:], prs_mini[:1, 0:1].bitcast(F32R),
                                 junk_row[:], start=True, stop=True)

    _split_excess_waits(nc)
    return nc


def make_in_maps(inputs):
    """Host-side shard/layout prep: slicing, transposes, concats, zero-pad
    only (no arithmetic)."""
    f = {k: np.ascontiguousarray(np.asarray(v, dtype=np.float32))
         for k, v in inputs.items()}

    wg_p = np.zeros((GP, H), np.float32)
    wg_p[:G] = f["w_gex"]
    wc_p = np.zeros((GP, H), np.float32)
    wc_p[:G] = f["w_comp"]
    ppi_p = np.zeros((GP, G), np.float32)
    ppi_p[:G] = f["ppi_adj"]

    aux_row = np.ascontiguousarray(
        np.stack([f["w_dose"][0], f["w_time"][0],
                  f["ln_gamma"], f["ln_beta"]]))

    in_maps = []
    for c in range(NCORES):
        sl = slice(c * BL, (c + 1) * BL)
        nfT = f["node_feat"][sl].transpose(2, 0, 1).reshape(FEAT, BL * NA)
        mix = np.zeros((52, 480), np.float32)
        mix[0:FEAT, 0:400] = nfT
        mix[0:FEAT, 400:464] = f["W1"]
        mix[0:NA, 464:472] = f["mask"][sl].T
        mix[0, 472:480] = f["dose"][sl, 0]
        mix[1, 472:480] = f["time"][sl, 0]
        m = {
            "b_gex": np.ascontiguousarray(f["b_gex"][sl]),
            "mix": mix,
            "adjT": np.ascontiguousarray(
                f["adj_matrix"][sl].transpose(1, 0, 2).reshape(NA, BL * NA)),
            "distT": np.ascontiguousarray(
                f["dist_matrix"][sl].transpose(1, 0, 2).reshape(NA, BL * NA)),
            "aux_row": aux_row,
            "ppi_slice": np.ascontiguousarray(ppi_p[c * 128:(c + 1) * 128]),
            "W2": f["W2"],
            "w_gex_p": wg_p,
            "w_comp_p": wc_p,
            "W_ff": f["W_ff"],
        }
        in_maps.append(m)
    return in_maps


def kernel(**inputs):
    nc = build_nc()
    in_maps = make_in_maps(inputs)
    r = run_bass_kernel_spmd(nc, in_maps, list(range(NCORES)))
    pred = np.concatenate([r.results[c]["out_pred"] for c in range(NCORES)], axis=0)
    comp = np.concatenate([r.results[c]["out_comp"] for c in range(NCORES)], axis=0)
    return pred, comp
